# revision 36
# baseline (speedup 1.0000x reference)
"""Trainium2 Bass kernel for the DCNv4 bottleneck block.

Contract: kernel(**inputs) takes FULL unsharded inputs (as in reference
setup_inputs()) and returns the FULL (4, 256, 80, 80) fp32 output.

Sharding: 8 cores = 4 samples x 2 row-halves (40 rows each + halos).

Per-core pipeline (channel-major [C-part, flat-pixel] on an 84-wide frame,
activations and matmuls in bf16, PSUM accumulation in fp32):
  cv1 3x3 conv (9 shifted matmuls) + BN + SiLU               -> y1 [128, 3696]
  cv2 1x1 conv + BN + SiLU                                   -> y  [2][128, 3712]
  depthwise 3x3 (diag matmuls) + LayerNorm + GELU            -> dw  [2][128, 3456]
  combined in/out projection Xc = (out_w@in_w) y + bias      -> xpm pixel-major
  offset/mask projection (pixel-major) -> bilinear/mask coefficients
  deformable sampling as banded matmul: M^T built by GPSIMD local_scatter,
  DMA-XBAR-transposed to M chunks, out[t, c] = sum_q M[q, t] Xpm[q, c]
  + SiLU (BN3 scale folded into the projection), store pixel-major.
  The residual add (x + z) happens on the host during unshard.
"""

import numpy as np
from contextlib import ExitStack

import ml_dtypes
import concourse.bass as bass
import concourse.tile as tile
from concourse import bacc, mybir
from concourse import bass_utils
from concourse.ap import AP

f32 = mybir.dt.float32
bf16 = mybir.dt.bfloat16
i16 = mybir.dt.int16
AF = mybir.ActivationFunctionType
OP = mybir.AluOpType
AX = mybir.AxisListType

# ---- geometry constants ----
W = 84                  # frame width (80 image + 2 pad each side)
HX, HY, HD = 46, 44, 40
NPX = HX * W            # 3864  x frame pixels
NPY = HY * W            # 3696  y frame pixels
YPAD = 3712             # y buffer (16 slack)
NPD = HD * W            # 3360  out-region pixels
NTOT = 3456             # padded out pixels (27 tiles)
NT = 27                 # out-pixel tiles of 128
LEAD = 130              # q = y-frame pixel + LEAD
XCH = 31                # Xpm chunks of 128
MTW = 512               # M^T row width (band cols p + 84u + v in [0, 468))
NCI = 4                 # M chunks per out tile; tile T reads xpm chunks T+1..T+4
PADIDX = 500            # scatter pad position (unused band column, zero value)
YOFF = 2                # y buffer head pad so stage C chunk Q reads 128(Q-1)
EPS_BN, EPS_LN = 1e-5, 1e-6
USE_DMA_TRANSPOSE = False

_BUILT = None


def _build(dump=False):
    nc = bacc.Bacc("TRN2", target_bir_lowering=False, debug=False, num_devices=8)

    # ---------------- DRAM I/O ----------------
    d_x = nc.dram_tensor("xs", [128, 2, NPX], bf16, kind="ExternalInput")
    d_w1t = nc.dram_tensor("w1t", [128, 9, 2, 128], bf16, kind="ExternalInput")
    d_w2t = nc.dram_tensor("w2t", [128, 256], bf16, kind="ExternalInput")
    d_wct = nc.dram_tensor("wct", [128, 2, 256], bf16, kind="ExternalInput")
    d_bc = nc.dram_tensor("bcr", [1, 256], bf16, kind="ExternalInput")
    d_dwdg = nc.dram_tensor("dwdg", [128, 2, 9, 128], bf16, kind="ExternalInput")
    d_dwb = nc.dram_tensor("dwb", [128, 2], f32, kind="ExternalInput")
    d_womt = nc.dram_tensor("womt", [128, 2, 32], bf16, kind="ExternalInput")
    d_omb = nc.dram_tensor("omb", [1, 32], bf16, kind="ExternalInput")
    d_s1 = nc.dram_tensor("s1", [128, 1], f32, kind="ExternalInput")
    d_t1 = nc.dram_tensor("t1", [128, 1], f32, kind="ExternalInput")
    d_s2 = nc.dram_tensor("s2", [128, 2], f32, kind="ExternalInput")
    d_t2 = nc.dram_tensor("t2", [128, 2], f32, kind="ExternalInput")
    d_lng = nc.dram_tensor("lng", [128, 2], f32, kind="ExternalInput")
    d_lnb = nc.dram_tensor("lnb", [128, 2], f32, kind="ExternalInput")
    d_ident = nc.dram_tensor("ident", [128, 128], f32, kind="ExternalInput")
    d_vmask = nc.dram_tensor("vmask", [128, XCH], f32, kind="ExternalInput")
    d_rowm = nc.dram_tensor("rowm", [128, 2], f32, kind="ExternalInput")
    d_t3r = nc.dram_tensor("t3r", [1, 256], bf16, kind="ExternalInput")
    d_onesr = nc.dram_tensor("onesr", [1, 512], bf16, kind="ExternalInput")
    d_onesc = nc.dram_tensor("onesc", [128, 1], bf16, kind="ExternalInput")
    d_identb = nc.dram_tensor("identb", [128, 128], bf16, kind="ExternalInput")
    d_out = nc.dram_tensor("out", [NT, 128, 256], f32, kind="ExternalOutput")
    d_rbs = nc.dram_tensor("rbs", [NT, 2, 128], bf16, kind="Internal")
    if dump:
        d_dy1 = nc.dram_tensor("dy1", [128, NPY], bf16, kind="ExternalOutput")
        d_dy = nc.dram_tensor("dy", [128, 2 * YPAD], bf16, kind="ExternalOutput")
        d_ddwg = nc.dram_tensor("ddwg", [128, 2 * NTOT], bf16, kind="ExternalOutput")
        d_dxpm = nc.dram_tensor("dxpm", [128, XCH * 256], bf16, kind="ExternalOutput")
        d_dcoef = nc.dram_tensor("dcoef", [128, NT * 32], f32, kind="ExternalOutput")
        d_dcbuf = nc.dram_tensor("dcbuf", [128, NT * 26], bf16, kind="ExternalOutput")
        d_drb = nc.dram_tensor("drb", [1, 2 * NTOT], bf16, kind="ExternalOutput")
        d_dstat = nc.dram_tensor("dstat", [128, NT * 2], f32, kind="ExternalOutput")

    with tile.TileContext(nc) as tc:
        with ExitStack() as ctx:
            P = ctx.enter_context(tc.tile_pool(name="persist", bufs=1))

            # ---------------- loads ----------------
            # HWDGE (sync/scalar) for all loads; x split so stage A starts early
            x_sb = P.tile([128, 2, NPX], bf16)
            xT = d_x.ap()
            xsplit = [0, NPX // 4, NPX // 2, 3 * NPX // 4, NPX]
            w1t = P.tile([128, 9, 2, 128], bf16)
            nc.scalar.dma_start(w1t[:], d_w1t.ap())
            for i in range(4):
                a, b = xsplit[i], xsplit[i + 1]
                eng = nc.sync if i % 2 == 0 else nc.scalar
                eng.dma_start(x_sb[:, :, a:b], xT[:, :, a:b])
            w2t = P.tile([128, 256], bf16)
            nc.gpsimd.dma_start(w2t[:], d_w2t.ap())
            wct = P.tile([128, 2, 256], bf16)
            nc.gpsimd.dma_start(wct[:], d_wct.ap())
            bcr = P.tile([1, 256], bf16)
            nc.gpsimd.dma_start(bcr[:], d_bc.ap())
            dwdg = P.tile([128, 2, 9, 128], bf16)
            nc.gpsimd.dma_start(dwdg[:], d_dwdg.ap())
            dwb = P.tile([128, 2], f32)
            nc.gpsimd.dma_start(dwb[:], d_dwb.ap())
            womt = P.tile([128, 2, 32], bf16)
            nc.gpsimd.dma_start(womt[:], d_womt.ap())
            omb = P.tile([1, 32], bf16)
            nc.gpsimd.dma_start(omb[:], d_omb.ap())
            s1 = P.tile([128, 1], f32)
            nc.sync.dma_start(s1[:], d_s1.ap())
            t1 = P.tile([128, 1], f32)
            nc.sync.dma_start(t1[:], d_t1.ap())
            s2 = P.tile([128, 2], f32)
            nc.gpsimd.dma_start(s2[:], d_s2.ap())
            t2 = P.tile([128, 2], f32)
            nc.gpsimd.dma_start(t2[:], d_t2.ap())
            lng = P.tile([128, 2], f32)
            nc.gpsimd.dma_start(lng[:], d_lng.ap())
            lnb = P.tile([128, 2], f32)
            nc.gpsimd.dma_start(lnb[:], d_lnb.ap())
            ident = P.tile([128, 128], f32)
            nc.gpsimd.dma_start(ident[:], d_ident.ap())
            identb = P.tile([128, 128], bf16)
            nc.gpsimd.dma_start(identb[:], d_identb.ap())
            vmask = P.tile([128, XCH], f32)
            nc.gpsimd.dma_start(vmask[:], d_vmask.ap())
            rowm = P.tile([128, 2], f32)
            nc.scalar.dma_start(rowm[:], d_rowm.ap())
            t3r = P.tile([1, 256], bf16)
            nc.gpsimd.dma_start(t3r[:], d_t3r.ap())
            ones_row = P.tile([1, 512], bf16)
            nc.scalar.dma_start(ones_row[:], d_onesr.ap())
            ones_col = P.tile([128, 1], bf16)
            nc.sync.dma_start(ones_col[:], d_onesc.ap())
            eps128 = P.tile([128, 1], f32)
            nc.vector.memset(eps128[:], EPS_LN)

            # scatter indices for M^T build: col = p + 84u + v, pad -> PADIDX
            sidx = P.tile([128, 26], i16)
            nc.gpsimd.iota(sidx[:, 0:25], pattern=[[84, 5], [1, 5]], base=0,
                           channel_multiplier=1, allow_small_or_imprecise_dtypes=True)
            nc.gpsimd.iota(sidx[:, 25:26], pattern=[[1, 1]], base=PADIDX,
                           channel_multiplier=0, allow_small_or_imprecise_dtypes=True)

            # broadcast bias tiles (built once via K=1 matmuls)
            t3b = P.tile([128, 256], f32)
            ombb = P.tile([128, 32], f32)
            with tc.tile_pool(name="psI", bufs=2, space="PSUM") as psI:
                for src, dst, nn in ((t3r, t3b, 256), (omb, ombb, 32)):
                    ps = psI.tile([128, 256], f32, tag="pi")
                    nc.tensor.matmul(ps[:, :nn], ones_row[:, 0:128], src[:],
                                     start=True, stop=True)
                    nc.vector.tensor_copy(dst[:], ps[:, :nn])

            # ---------------- persistent activations ----------------
            y1pool = tc.alloc_tile_pool(name="y1pool", bufs=1)
            y1 = y1pool.tile([128, NPY], bf16)
            y = P.tile([128, 2, YPAD], bf16)
            dw = P.tile([128, 2, NTOT], bf16)       # later overwritten by gelu output
            xpm = P.tile([128, XCH, 256], bf16)
            coefb = P.tile([128, NT, 32], f32)
            cbuf = P.tile([128, NT, 26], f32)
            cbufh = P.tile([128, NT, 26], bf16)
            statb = P.tile([128, NT, 2], f32)

            # =============== stage A: cv1 ===============
            with tc.tile_pool(name="psA", bufs=3, space="PSUM") as psA:
                t0 = 1
                while t0 < NPY - 1:
                    nn = min(512, NPY - 1 - t0)
                    ps = psA.tile([128, 512], f32)
                    first = True
                    for ck in range(2):
                        for s in range(9):
                            ki, kj = s // 3, s % 3
                            off = ki * W + kj - 1
                            nc.tensor.matmul(
                                ps[:, :nn], w1t[:, s, ck, :],
                                x_sb[:, ck, t0 + off: t0 + off + nn],
                                start=first, stop=(ck == 1 and s == 8))
                            first = False
                    nc.scalar.activation(y1[:, t0:t0 + nn], ps[:, :nn], AF.Silu,
                                         bias=t1[:], scale=s1[:])
                    t0 += nn
            nc.vector.memset(y1[:, 0:1], 0.0)
            nc.vector.memset(y1[:, NPY - 1:NPY], 0.0)

            # =============== stage B: cv2 ===============
            with tc.tile_pool(name="psB", bufs=3, space="PSUM") as psB:
                t0 = 0
                while t0 < NPY:
                    nn = min(512, NPY - t0)
                    for m in range(2):
                        ps = psB.tile([128, 512], f32)
                        nc.tensor.matmul(ps[:, :nn], w2t[:, 128 * m:128 * m + 128],
                                         y1[:, t0:t0 + nn], start=True, stop=True)
                        nc.scalar.activation(y[:, m, YOFF + t0:YOFF + t0 + nn],
                                             ps[:, :nn], AF.Silu,
                                             bias=t2[:, m:m + 1], scale=s2[:, m:m + 1])
                    t0 += nn
            y1pool.release()
            # zero the y head/tail and pad columns; zero out-of-image rows via rowmask
            for m in range(2):
                nc.vector.memset(y[:, m, 0:YOFF], 0.0)
                nc.vector.memset(y[:, m, YOFF + NPY:YPAD], 0.0)
                yv = AP(y.tensor, y[:].offset + m * YPAD + YOFF,
                        [[2 * YPAD, 128], [W, HY], [1, 2]])
                nc.vector.memset(yv, 0.0)
                yv2 = AP(y.tensor, y[:].offset + m * YPAD + YOFF + 82,
                         [[2 * YPAD, 128], [W, HY], [1, 2]])
                nc.vector.memset(yv2, 0.0)
                nc.vector.tensor_scalar(y[:, m, YOFF:YOFF + 2 * W],
                                        y[:, m, YOFF:YOFF + 2 * W],
                                        rowm[:, 0:1], None, OP.mult)
                nc.vector.tensor_scalar(y[:, m, YOFF + NPY - 2 * W:YOFF + NPY],
                                        y[:, m, YOFF + NPY - 2 * W:YOFF + NPY],
                                        rowm[:, 1:2], None, OP.mult)

            # =============== stage D: depthwise conv + LN + GELU ===============
            # D1: depthwise conv (diag matmuls) + per-pixel channel sums /
            # sumsq via N=1 matmuls (pixel-major stats on 128 lanes).
            with tc.tile_pool(name="psD", bufs=3, space="PSUM") as psD, \
                 tc.tile_pool(name="psS", bufs=2, space="PSUM") as psS, \
                 tc.tile_pool(name="dtmp", bufs=2) as dtmp:
                t0 = 0
                while t0 < NTOT:
                    nn = min(512, NTOT - t0)
                    sc = nn // 128
                    sq = [None, None]
                    for m in range(2):
                        ps = psD.tile([128, 512], f32, tag="pdw")
                        for ss in range(9):
                            ki, kj = ss // 3, ss % 3
                            off = (ki + 1) * W + kj - 1 + YOFF
                            nc.tensor.matmul(ps[:, :nn], dwdg[:, m, ss, :],
                                             y[:, m, t0 + off: t0 + off + nn],
                                             start=(ss == 0), stop=(ss == 8))
                        nc.vector.tensor_scalar(dw[:, m, t0:t0 + nn], ps[:, :nn],
                                                dwb[:, m:m + 1], None, OP.add)
                        sqm = dtmp.tile([128, 512], bf16, tag=f"sq{m}")
                        nc.gpsimd.tensor_tensor(sqm[:, :nn], dw[:, m, t0:t0 + nn],
                                                dw[:, m, t0:t0 + nn], OP.mult)
                        sq[m] = sqm
                    pst = psS.tile([128, 8], f32, tag="pstat")
                    for sub in range(sc):
                        sl = slice(t0 + 128 * sub, t0 + 128 * sub + 128)
                        for m in range(2):
                            nc.tensor.matmul(pst[:, 2 * sub:2 * sub + 1], dw[:, m, sl],
                                             ones_col[:], start=(m == 0), stop=(m == 1))
                            nc.tensor.matmul(pst[:, 2 * sub + 1:2 * sub + 2],
                                             sq[m][:, 128 * sub:128 * sub + 128],
                                             ones_col[:], start=(m == 0), stop=(m == 1))
                    nc.vector.tensor_copy(statb[:, t0 // 128: t0 // 128 + sc, :],
                                          pst[:, :2 * sc])
                    t0 += nn

            # D2: stats math on [128, NT] (all lanes), PE transpose to [NT, 128],
            # then a DRAM bounce to repack as rows [1, 2, NTOT] (rstd | brow).
            rb_row = P.tile([1, 2, NTOT], bf16)
            with tc.tile_pool(name="stt", bufs=1) as sttp, \
                 tc.tile_pool(name="psST", bufs=2, space="PSUM") as psST:
                st0 = AP(statb.tensor, statb[:].offset, [[NT * 2, 128], [2, NT]])
                st1 = AP(statb.tensor, statb[:].offset + 1, [[NT * 2, 128], [2, NT]])
                meanb = sttp.tile([128, NT], f32)
                nc.vector.tensor_scalar(meanb[:], st0, 1.0 / 256, None, OP.mult)
                ex2 = sttp.tile([128, NT], f32)
                nc.vector.tensor_scalar(ex2[:], st1, 1.0 / 256, None, OP.mult)
                msq = sttp.tile([128, NT], f32)
                nc.vector.tensor_tensor(msq[:], meanb[:], meanb[:], OP.mult)
                nc.vector.tensor_tensor(ex2[:], ex2[:], msq[:], OP.subtract)
                sdev = sttp.tile([128, NT], f32)
                nc.scalar.activation(sdev[:], ex2[:], AF.Sqrt, bias=eps128[:], scale=1.0)
                rstdb = sttp.tile([128, NT], f32)
                with nc.allow_low_precision(reason="LN rstd"):
                    nc.vector.reciprocal(rstdb[:], sdev[:])
                browb = sttp.tile([128, NT], f32)
                nc.vector.scalar_tensor_tensor(browb[:], meanb[:], -1.0, rstdb[:],
                                               OP.mult, OP.mult)
                rbT = sttp.tile([32, 256], bf16)
                for src, col0 in ((rstdb, 0), (browb, 128)):
                    pT = psST.tile([128, 128], f32, tag="pT")
                    nc.tensor.transpose(pT[:NT, :], src[:], ident[:])
                    nc.vector.tensor_copy(rbT[:NT, col0:col0 + 128], pT[:NT, :])
                st_dma = nc.sync.dma_start(
                    d_rbs.ap(), rbT[:NT, :].rearrange("p (a b) -> p a b", a=2))
                ld_dma = nc.sync.dma_start(
                    rb_row[:].rearrange("o a (b c) -> o a b c", b=NT),
                    d_rbs.ap().transpose([1, 0, 2]).unsqueeze(0))
                tile.add_dep_helper(ld_dma.ins, st_dma.ins,
                                    reason="dram bounce ordering")

            # D3: normalize + gelu (per-pixel rstd/brow broadcast via K=1 matmul)
            with tc.tile_pool(name="psAB", bufs=2, space="PSUM") as psAB, \
                 tc.tile_pool(name="dtmp2", bufs=2) as dtmp2:
                t0 = 0
                while t0 < NTOT:
                    nn = min(512, NTOT - t0)
                    psa = psAB.tile([128, 512], f32, tag="pa")
                    nc.tensor.matmul(psa[:, :nn], ones_row[:, 0:128],
                                     rb_row[:, 0, t0:t0 + nn], start=True, stop=True)
                    psb = psAB.tile([128, 512], f32, tag="pb")
                    nc.tensor.matmul(psb[:, :nn], ones_row[:, 0:128],
                                     rb_row[:, 1, t0:t0 + nn], start=True, stop=True)
                    for m in range(2):
                        zt = dtmp2.tile([128, 512], bf16, tag=f"zt{m}")
                        nc.vector.tensor_tensor(zt[:, :nn], dw[:, m, t0:t0 + nn],
                                                psa[:, :nn], OP.mult)
                        nc.vector.tensor_tensor(zt[:, :nn], zt[:, :nn], psb[:, :nn], OP.add)
                        nc.scalar.activation(dw[:, m, t0:t0 + nn], zt[:, :nn],
                                             AF.Gelu, bias=lnb[:, m:m + 1], scale=lng[:, m:m + 1])
                    t0 += nn

            # =============== stage C: Xc projection -> xpm (pixel-major) ===============
            # (placed after D3 so its PE work fills the DVE-bound middle region)
            nc.vector.memset(xpm[:, 0, :], 0.0)
            nc.vector.memset(xpm[:, XCH - 1, :], 0.0)
            with tc.tile_pool(name="psC", bufs=3, space="PSUM") as psC:
                for Q in range(1, XCH - 1):
                    p0 = 128 * (Q - 1)
                    ps = psC.tile([128, 256], f32)
                    nc.tensor.matmul(ps[:], y[:, 0, p0:p0 + 128], wct[:, 0, :],
                                     start=True, stop=False)
                    nc.tensor.matmul(ps[:], y[:, 1, p0:p0 + 128], wct[:, 1, :],
                                     start=False, stop=False)
                    nc.tensor.matmul(ps[:], ones_row[:, 0:128], bcr[:],
                                     start=False, stop=True)
                    nc.vector.tensor_scalar(xpm[:, Q, :], ps[:],
                                            vmask[:, Q:Q + 1], None, OP.mult)

            # =============== stage E: offset/mask projection + coefficients ===============
            with tc.tile_pool(name="psE", bufs=3, space="PSUM") as psE:
                for T in range(NT):
                    ps = psE.tile([128, 32], f32)
                    nc.tensor.matmul(ps[:], dw[:, 0, 128 * T:128 * T + 128], womt[:, 0, :],
                                     start=True, stop=False)
                    nc.tensor.matmul(ps[:], dw[:, 1, 128 * T:128 * T + 128], womt[:, 1, :],
                                     start=False, stop=True)
                    nc.vector.tensor_tensor(coefb[:, T, :], ps[:], ombb[:], OP.add)

            # coefficient math, vectorized over all tiles: views [128, NT, 9]
            cf = coefb[:].offset
            cten = coefb.tensor

            def cview(col0, step, cnt=9):
                return AP(cten, cf + col0, [[NT * 32, 128], [32, NT], [step, cnt]])

            ox = cview(0, 2)
            oy = cview(1, 2)
            lg = cview(18, 1)
            with tc.tile_pool(name="cf", bufs=1) as cfp:
                mx = cfp.tile([128, NT], f32)
                nc.vector.tensor_reduce(mx[:], lg, axis=AX.X, op=OP.max)
                E = cfp.tile([128, NT, 9], f32)
                mxb = AP(mx.tensor, mx[:].offset, [[NT, 128], [1, NT], [0, 9]])
                nc.vector.tensor_tensor(E[:], lg, mxb, OP.subtract)
                nc.scalar.activation(E[:], E[:], AF.Exp)
                se = cfp.tile([128, NT], f32)
                nc.vector.tensor_reduce(se[:], E[:], axis=AX.X, op=OP.add)
                rs = cfp.tile([128, NT], f32)
                nc.vector.reciprocal(rs[:], se[:])
                msm = cfp.tile([128, NT, 9], f32)
                rsb = AP(rs.tensor, rs[:].offset, [[NT, 128], [1, NT], [0, 9]])
                nc.vector.tensor_tensor(msm[:], E[:], rsb, OP.mult)
                # fractional parts and floor indicators
                ix = cfp.tile([128, NT, 9], f32)
                nc.vector.tensor_scalar(ix[:], ox, 0.0, None, OP.is_lt)
                iy = cfp.tile([128, NT, 9], f32)
                nc.vector.tensor_scalar(iy[:], oy, 0.0, None, OP.is_lt)
                lx = cfp.tile([128, NT, 9], f32)
                nc.vector.tensor_tensor(lx[:], ox, ix[:], OP.add)
                ly = cfp.tile([128, NT, 9], f32)
                nc.vector.tensor_tensor(ly[:], oy, iy[:], OP.add)
                wx0 = cfp.tile([128, NT, 9], f32)
                nc.vector.tensor_scalar(wx0[:], lx[:], -1.0, 1.0, OP.mult, OP.add)
                wy0 = cfp.tile([128, NT, 9], f32)
                nc.vector.tensor_scalar(wy0[:], ly[:], -1.0, 1.0, OP.mult, OP.add)
                mx0 = cfp.tile([128, NT, 9], f32)
                nc.vector.tensor_scalar(mx0[:], ix[:], -1.0, 1.0, OP.mult, OP.add)
                my0 = cfp.tile([128, NT, 9], f32)
                nc.vector.tensor_scalar(my0[:], iy[:], -1.0, 1.0, OP.mult, OP.add)
                ta = cfp.tile([128, NT, 9], f32)
                nc.vector.tensor_tensor(ta[:], msm[:], wy0[:], OP.mult)
                tb = cfp.tile([128, NT, 9], f32)
                nc.vector.tensor_tensor(tb[:], msm[:], ly[:], OP.mult)
                pab = []
                for a, tv in ((0, ta), (1, tb)):
                    for b, wv in ((0, wx0), (1, lx)):
                        pv = cfp.tile([128, NT, 9], f32, name=f"p{a}{b}")
                        nc.vector.tensor_tensor(pv[:], tv[:], wv[:], OP.mult)
                        pab.append((a, b, pv))
                nc.vector.memset(cbuf[:], 0.0)
                gt = cfp.tile([128, NT, 9], f32)
                contrib = cfp.tile([128, NT, 9], f32)
                for sy, myv in ((0, my0), (1, iy)):
                    for sx, mxv in ((0, mx0), (1, ix)):
                        nc.vector.tensor_tensor(gt[:], myv[:], mxv[:], OP.mult)
                        for a, b, pv in pab:
                            u0 = 1 + a - sy
                            v0 = 1 + b - sx
                            nc.vector.tensor_tensor(contrib[:], pv[:], gt[:], OP.mult)
                            # C5[:, :, u0 + j, v0 + i] += contrib[i, j]
                            dstv = AP(cbuf.tensor, cbuf[:].offset + (u0 * 5 + v0),
                                      [[NT * 26, 128], [26, NT], [1, 3], [5, 3]])
                            srcv = AP(contrib.tensor, contrib[:].offset,
                                      [[NT * 9, 128], [9, NT], [3, 3], [1, 3]])
                            nc.vector.tensor_tensor(dstv, dstv, srcv, OP.add)
                nc.vector.tensor_copy(cbufh[:], cbuf[:])

            # =============== stage F: sampling + finalize ===============
            with tc.tile_pool(name="mtp", bufs=3) as mtp, \
                 tc.tile_pool(name="msb", bufs=3) as msbp, \
                 tc.tile_pool(name="psT", bufs=2, space="PSUM") as psT, \
                 tc.tile_pool(name="psZ", bufs=2, space="PSUM") as psZ, \
                 tc.tile_pool(name="fin", bufs=3) as fin:
                for T in range(NT):
                    mt = mtp.tile([128, MTW], bf16, tag="mt")
                    nc.gpsimd.local_scatter(mt[:].bitcast(i16), cbufh[:, T, :].bitcast(i16),
                                            sidx[:], channels=128, num_elems=MTW,
                                            num_idxs=26)
                    msb = msbp.tile([128, NCI, 128], bf16, tag="msb")
                    if USE_DMA_TRANSPOSE:
                        eng = nc.sync if T % 2 == 0 else nc.scalar
                        eng.dma_start(msb[:], mt[:], transpose=True)
                    else:
                        for ci in range(NCI):
                            pst = psT.tile([128, 128], bf16, tag="pst")
                            nc.tensor.transpose(pst[:], mt[:, 128 * ci:128 * ci + 128],
                                                identb[:])
                            if ci % 2 == 0:
                                nc.vector.tensor_copy(msb[:, ci, :], pst[:])
                            else:
                                nc.scalar.copy(msb[:, ci, :], pst[:])
                    psz = psZ.tile([128, 256], f32, tag="psz")
                    for ci in range(NCI):
                        nc.tensor.matmul(psz[:], msb[:, ci, :], xpm[:, T + 1 + ci, :],
                                         start=(ci == 0), stop=(ci == NCI - 1))
                    zt3 = fin.tile([128, 256], f32, tag="zt3")
                    nc.vector.tensor_tensor(zt3[:], psz[:], t3b[:], OP.add)
                    osb = fin.tile([128, 256], f32, tag="osb")
                    nc.scalar.activation(osb[:], zt3[:], AF.Silu)
                    nc.sync.dma_start(d_out.ap()[T], osb[:])

            if dump:
                nc.sync.dma_start(d_dy.ap(), y[:].rearrange("p a b -> p (a b)"))
                nc.sync.dma_start(d_ddwg.ap(), dw[:].rearrange("p a b -> p (a b)"))
                nc.sync.dma_start(d_dxpm.ap(), xpm[:].rearrange("p a b -> p (a b)"))
                nc.sync.dma_start(d_dcoef.ap(), coefb[:].rearrange("p a b -> p (a b)"))
                nc.sync.dma_start(d_dcbuf.ap(), cbufh[:].rearrange("p a b -> p (a b)"))
                nc.sync.dma_start(d_drb.ap(), rb_row[:].rearrange("o a b -> o (a b)"))
                nc.sync.dma_start(d_dstat.ap(), statb[:].rearrange("p a b -> p (a b)"))

    nc.compile()
    return nc


def _get_built():
    global _BUILT
    if _BUILT is None:
        _BUILT = _build()
    return _BUILT


def _prep(inputs):
    g = {k: np.asarray(v, dtype=np.float32) for k, v in inputs.items()}
    x = g["x"]
    b16 = ml_dtypes.bfloat16

    s1 = g["g1"] / np.sqrt(g["v1"] + EPS_BN)
    t1 = g["b1"] - g["m1"] * s1
    s2 = g["g2"] / np.sqrt(g["v2"] + EPS_BN)
    t2 = g["b2"] - g["m2"] * s2
    s3 = g["g3"] / np.sqrt(g["v3"] + EPS_BN)
    t3 = g["b3"] - g["m3"] * s3

    w1 = g["w1"]  # [128, 256, 3, 3]
    w1t = np.zeros((9, 2, 128, 128), np.float32)
    for ki in range(3):
        for kj in range(3):
            for ck in range(2):
                w1t[ki * 3 + kj, ck] = w1[:, 128 * ck:128 * ck + 128, ki, kj].T
    w2t = g["w2"][:, :, 0, 0].T.copy()  # [128, 256]
    Wc = g["out_w"] @ g["in_w"]
    wct = np.stack([Wc.T[:128], Wc.T[128:]])  # [2, 128, 256]
    bc = (g["out_w"] @ g["in_b"] + g["out_b"])[None, :]  # [1, 256]
    dwdg = np.zeros((2, 9, 128, 128), np.float32)
    for ck in range(2):
        for s in range(9):
            np.fill_diagonal(dwdg[ck, s], g["dw_w"][128 * ck:128 * ck + 128, 0, s // 3, s % 3])
    dwb = np.ascontiguousarray(g["dw_b"].reshape(2, 128).T).astype(np.float32)
    womt = np.zeros((2, 128, 32), np.float32)
    for ck in range(2):
        womt[ck, :, :18] = g["off_w"][:, 128 * ck:128 * ck + 128].T
        womt[ck, :, 18:27] = g["msk_w"][:, 128 * ck:128 * ck + 128].T
    omb = np.zeros((1, 32), np.float32)
    omb[0, :18] = g["off_b"]
    omb[0, 18:27] = g["msk_b"]
    ident = np.eye(128, dtype=np.float32)

    def colsplit(v):  # [256] -> [128, 2]
        return np.ascontiguousarray(v.reshape(2, 128).T).astype(np.float32)

    # fold BN3 scale into the combined projection; t3 added on-chip
    wct = (wct.reshape(2, 128, 256) * s3[None, None, :]).astype(np.float32)
    bc = (bc * s3[None, :]).astype(np.float32)

    shared = dict(
        onesr=np.ones((1, 512), b16),
        onesc=np.ones((128, 1), b16),
        w1t=np.ascontiguousarray(w1t.transpose(2, 0, 1, 3)).astype(b16),
        w2t=w2t.astype(b16),
        wct=np.ascontiguousarray(wct.transpose(1, 0, 2)).astype(b16),
        bcr=bc.astype(b16),
        dwdg=np.ascontiguousarray(dwdg.transpose(2, 0, 1, 3)).astype(b16),
        dwb=dwb,
        womt=np.ascontiguousarray(womt.transpose(1, 0, 2)).astype(b16),
        omb=omb.astype(b16),
        s1=s1[:, None], t1=t1[:, None],
        s2=colsplit(s2), t2=colsplit(t2),
        lng=colsplit(g["ln_g"]), lnb=colsplit(g["ln_b"]), ident=ident,
        identb=ident.astype(b16),
        t3r=t3[None, :].astype(b16),
    )

    in_maps = []
    for c in range(8):
        n, h = c // 2, c % 2
        r0 = 40 * h - 3  # x frame row 0 in global coords
        xs = np.zeros((2, 128, HX, W), np.float32)
        glo = max(r0, 0)
        ghi = min(r0 + HX, 80)
        xs[0, :, glo - r0:ghi - r0, 2:82] = x[n, :128, glo:ghi, :]
        xs[1, :, glo - r0:ghi - r0, 2:82] = x[n, 128:, glo:ghi, :]
        # validity mask for xpm pixels: q = 128*Q + p, pix = q - LEAD
        vm = np.zeros((XCH * 128,), np.float32)
        qs = np.arange(XCH * 128)
        pix = qs - LEAD
        rv, cv = pix // W, pix % W
        gr = 40 * h + rv - 2
        ok = (pix >= 0) & (pix < NPY) & (cv >= 2) & (cv < 82) & (gr >= 0) & (gr < 80)
        vm[ok] = 1.0
        vmask = vm.reshape(XCH, 128).T.copy()  # [128, XCH]
        rowm = np.zeros((128, 2), np.float32)
        rowm[:, 0] = 0.0 if h == 0 else 1.0   # y rows [0,2) valid only for h=1
        rowm[:, 1] = 1.0 if h == 0 else 0.0   # y rows [42,44) valid only for h=0
        m = dict(shared)
        m["xs"] = np.ascontiguousarray(
            xs.reshape(2, 128, NPX).transpose(1, 0, 2)).astype(b16)
        m["vmask"] = vmask
        m["rowm"] = rowm
        in_maps.append(m)
    return in_maps


def kernel(**inputs):
    nc = _get_built()
    in_maps = _prep(inputs)
    res = bass_utils.run_bass_kernel_spmd(nc, in_maps, core_ids=list(range(8)))
    x = np.asarray(inputs["x"], np.float32)
    out = np.zeros((4, 256, 80, 80), np.float32)
    for c in range(8):
        n, h = c // 2, c % 2
        o = np.asarray(res.results[c]["out"], np.float32).reshape(NT * 128, 256)[:NPD]
        o = o.reshape(HD, W, 256)[:, 2:82].transpose(2, 0, 1)
        out[n, :, 40 * h:40 * h + 40, :] = o
    out += x
    return out


# revision 38
# speedup vs baseline: 1.1527x; 1.1527x over previous
"""Trainium2 Bass kernel for the DCNv4 bottleneck block.

Contract: kernel(**inputs) takes FULL unsharded inputs (as in reference
setup_inputs()) and returns the FULL (4, 256, 80, 80) fp32 output.

Sharding: 8 cores = 4 samples x 2 row-halves (40 rows each + halos).

Per-core pipeline (channel-major [C-part, flat-pixel] on an 84-wide frame,
activations and matmuls in bf16, PSUM accumulation in fp32):
  cv1 3x3 conv (9 shifted matmuls) + BN + SiLU               -> y1 [128, 3696]
  cv2 1x1 conv + BN + SiLU                                   -> y  [2][128, 3712]
  depthwise 3x3 (diag matmuls) + LayerNorm + GELU            -> dw  [2][128, 3456]
  combined in/out projection Xc = (out_w@in_w) y + bias      -> xpm pixel-major
  offset/mask projection (pixel-major) -> bilinear/mask coefficients
  deformable sampling as banded matmul: M^T built by GPSIMD local_scatter,
  DMA-XBAR-transposed to M chunks, out[t, c] = sum_q M[q, t] Xpm[q, c]
  + SiLU (BN3 scale folded into the projection), store pixel-major.
  The residual add (x + z) happens on the host during unshard.
"""

import numpy as np
from contextlib import ExitStack

import ml_dtypes
import concourse.bass as bass
import concourse.tile as tile
from concourse import bacc, mybir
from concourse import bass_utils
from concourse.ap import AP

f32 = mybir.dt.float32
bf16 = mybir.dt.bfloat16
i16 = mybir.dt.int16
AF = mybir.ActivationFunctionType
OP = mybir.AluOpType
AX = mybir.AxisListType

# ---- geometry constants ----
W = 84                  # frame width (80 image + 2 pad each side)
HX, HY, HD = 46, 44, 40
NPX = HX * W            # 3864  x frame pixels
NPY = HY * W            # 3696  y frame pixels
YPAD = 3712             # y buffer (16 slack)
NPD = HD * W            # 3360  out-region pixels
NTOT = 3456             # padded out pixels (27 tiles)
NT = 27                 # out-pixel tiles of 128
LEAD = 130              # q = y-frame pixel + LEAD
XCH = 31                # Xpm chunks of 128
MTW = 512               # M^T row width (band cols p + 84u + v in [0, 468))
NCI = 4                 # M chunks per out tile; tile T reads xpm chunks T+1..T+4
PADIDX = 500            # scatter pad position (unused band column, zero value)
YOFF = 2                # y buffer head pad so stage C chunk Q reads 128(Q-1)
EPS_BN, EPS_LN = 1e-5, 1e-6
USE_DMA_TRANSPOSE = False

_BUILT = None


def _build(dump=False):
    nc = bacc.Bacc("TRN2", target_bir_lowering=False, debug=False, num_devices=8)

    # ---------------- DRAM I/O ----------------
    d_x = nc.dram_tensor("xs", [128, 2, NPX], bf16, kind="ExternalInput")
    d_w1t = nc.dram_tensor("w1t", [128, 9, 2, 128], bf16, kind="ExternalInput")
    d_w2t = nc.dram_tensor("w2t", [128, 256], bf16, kind="ExternalInput")
    d_wct = nc.dram_tensor("wct", [128, 2, 256], bf16, kind="ExternalInput")
    d_bc = nc.dram_tensor("bcr", [1, 256], bf16, kind="ExternalInput")
    d_dwdg = nc.dram_tensor("dwdg", [128, 2, 9, 128], bf16, kind="ExternalInput")
    d_dwb = nc.dram_tensor("dwb", [128, 2], f32, kind="ExternalInput")
    d_womt = nc.dram_tensor("womt", [128, 2, 32], bf16, kind="ExternalInput")
    d_omb = nc.dram_tensor("omb", [1, 32], bf16, kind="ExternalInput")
    d_s1 = nc.dram_tensor("s1", [128, 1], f32, kind="ExternalInput")
    d_t1 = nc.dram_tensor("t1", [128, 1], f32, kind="ExternalInput")
    d_s2 = nc.dram_tensor("s2", [128, 2], f32, kind="ExternalInput")
    d_t2 = nc.dram_tensor("t2", [128, 2], f32, kind="ExternalInput")
    d_lng = nc.dram_tensor("lng", [128, 2], f32, kind="ExternalInput")
    d_lnb = nc.dram_tensor("lnb", [128, 2], f32, kind="ExternalInput")
    d_ident = nc.dram_tensor("ident", [128, 128], f32, kind="ExternalInput")
    d_vmask = nc.dram_tensor("vmask", [128, XCH], f32, kind="ExternalInput")
    d_rowm = nc.dram_tensor("rowm", [128, 2], f32, kind="ExternalInput")
    d_t3r = nc.dram_tensor("t3r", [1, 256], bf16, kind="ExternalInput")
    d_onesr = nc.dram_tensor("onesr", [1, 512], bf16, kind="ExternalInput")
    d_onesc = nc.dram_tensor("onesc", [128, 1], bf16, kind="ExternalInput")
    d_identb = nc.dram_tensor("identb", [128, 128], bf16, kind="ExternalInput")
    d_out = nc.dram_tensor("out", [NT, 128, 256], f32, kind="ExternalOutput")
    d_rbs = nc.dram_tensor("rbs", [NT, 2, 128], bf16, kind="Internal")
    if dump:
        d_dy1 = nc.dram_tensor("dy1", [128, NPY], bf16, kind="ExternalOutput")
        d_dy = nc.dram_tensor("dy", [128, 2 * YPAD], bf16, kind="ExternalOutput")
        d_ddwg = nc.dram_tensor("ddwg", [128, 2 * NTOT], bf16, kind="ExternalOutput")
        d_dxpm = nc.dram_tensor("dxpm", [128, XCH * 256], bf16, kind="ExternalOutput")
        d_dcoef = nc.dram_tensor("dcoef", [128, NT * 32], f32, kind="ExternalOutput")
        d_dcbuf = nc.dram_tensor("dcbuf", [128, NT * 26], bf16, kind="ExternalOutput")
        d_drb = nc.dram_tensor("drb", [1, 2 * NTOT], bf16, kind="ExternalOutput")
        d_dstat = nc.dram_tensor("dstat", [128, NT * 2], f32, kind="ExternalOutput")

    with tile.TileContext(nc) as tc:
        with ExitStack() as ctx:
            P = ctx.enter_context(tc.tile_pool(name="persist", bufs=1))

            # ---------------- loads ----------------
            # HWDGE (sync/scalar) for all loads; x split so stage A starts early
            x_sb = P.tile([128, 2, NPX], bf16)
            xT = d_x.ap()
            xsplit = [0, NPX // 4, NPX // 2, 3 * NPX // 4, NPX]
            w1t = P.tile([128, 9, 2, 128], bf16)
            nc.scalar.dma_start(w1t[:], d_w1t.ap())
            for i in range(4):
                a, b = xsplit[i], xsplit[i + 1]
                eng = nc.sync if i % 2 == 0 else nc.scalar
                eng.dma_start(x_sb[:, :, a:b], xT[:, :, a:b])
            w2t = P.tile([128, 256], bf16)
            nc.gpsimd.dma_start(w2t[:], d_w2t.ap())
            wct = P.tile([128, 2, 256], bf16)
            nc.gpsimd.dma_start(wct[:], d_wct.ap())
            bcr = P.tile([1, 256], bf16)
            nc.gpsimd.dma_start(bcr[:], d_bc.ap())
            dwdg = P.tile([128, 2, 9, 128], bf16)
            nc.gpsimd.dma_start(dwdg[:], d_dwdg.ap())
            dwb = P.tile([128, 2], f32)
            nc.gpsimd.dma_start(dwb[:], d_dwb.ap())
            womt = P.tile([128, 2, 32], bf16)
            nc.gpsimd.dma_start(womt[:], d_womt.ap())
            omb = P.tile([1, 32], bf16)
            nc.gpsimd.dma_start(omb[:], d_omb.ap())
            s1 = P.tile([128, 1], f32)
            nc.sync.dma_start(s1[:], d_s1.ap())
            t1 = P.tile([128, 1], f32)
            nc.sync.dma_start(t1[:], d_t1.ap())
            s2 = P.tile([128, 2], f32)
            nc.gpsimd.dma_start(s2[:], d_s2.ap())
            t2 = P.tile([128, 2], f32)
            nc.gpsimd.dma_start(t2[:], d_t2.ap())
            lng = P.tile([128, 2], f32)
            nc.gpsimd.dma_start(lng[:], d_lng.ap())
            lnb = P.tile([128, 2], f32)
            nc.gpsimd.dma_start(lnb[:], d_lnb.ap())
            ident = P.tile([128, 128], f32)
            nc.gpsimd.dma_start(ident[:], d_ident.ap())
            identb = P.tile([128, 128], bf16)
            nc.gpsimd.dma_start(identb[:], d_identb.ap())
            vmask = P.tile([128, XCH], f32)
            nc.gpsimd.dma_start(vmask[:], d_vmask.ap())
            rowm = P.tile([128, 2], f32)
            nc.scalar.dma_start(rowm[:], d_rowm.ap())
            t3r = P.tile([1, 256], bf16)
            nc.gpsimd.dma_start(t3r[:], d_t3r.ap())
            ones_row = P.tile([1, 512], bf16)
            nc.scalar.dma_start(ones_row[:], d_onesr.ap())
            ones_col = P.tile([128, 1], bf16)
            nc.sync.dma_start(ones_col[:], d_onesc.ap())
            eps128 = P.tile([128, 1], f32)
            nc.vector.memset(eps128[:], EPS_LN)

            # scatter indices for M^T build: col = p + 84u + v, pad -> PADIDX
            sidx = P.tile([128, 26], i16)
            nc.gpsimd.iota(sidx[:, 0:25], pattern=[[84, 5], [1, 5]], base=0,
                           channel_multiplier=1, allow_small_or_imprecise_dtypes=True)
            nc.gpsimd.iota(sidx[:, 25:26], pattern=[[1, 1]], base=PADIDX,
                           channel_multiplier=0, allow_small_or_imprecise_dtypes=True)

            # broadcast bias tiles (built once via K=1 matmuls)
            t3b = P.tile([128, 256], f32)
            ombb = P.tile([128, 32], f32)
            with tc.tile_pool(name="psI", bufs=2, space="PSUM") as psI:
                for src, dst, nn in ((t3r, t3b, 256), (omb, ombb, 32)):
                    ps = psI.tile([128, 256], f32, tag="pi")
                    nc.tensor.matmul(ps[:, :nn], ones_row[:, 0:128], src[:],
                                     start=True, stop=True)
                    nc.vector.tensor_copy(dst[:], ps[:, :nn])

            # ---------------- persistent activations ----------------
            y1pool = tc.alloc_tile_pool(name="y1pool", bufs=1)
            y1 = y1pool.tile([128, NPY], bf16)
            y = P.tile([128, 2, YPAD], bf16)
            dw = P.tile([128, 2, NTOT], bf16)       # later overwritten by gelu output
            xpm = P.tile([128, XCH, 256], bf16)
            coefb = P.tile([128, NT, 32], f32)
            cbuf = P.tile([128, NT, 26], f32)
            cbufh = P.tile([128, NT, 26], bf16)
            statb = P.tile([128, NT, 2], f32)

            # =============== stage A: cv1 ===============
            with tc.tile_pool(name="psA", bufs=3, space="PSUM") as psA:
                t0 = 1
                while t0 < NPY - 1:
                    nn = min(512, NPY - 1 - t0)
                    ps = psA.tile([128, 512], f32)
                    first = True
                    for ck in range(2):
                        for s in range(9):
                            ki, kj = s // 3, s % 3
                            off = ki * W + kj - 1
                            nc.tensor.matmul(
                                ps[:, :nn], w1t[:, s, ck, :],
                                x_sb[:, ck, t0 + off: t0 + off + nn],
                                start=first, stop=(ck == 1 and s == 8))
                            first = False
                    nc.scalar.activation(y1[:, t0:t0 + nn], ps[:, :nn], AF.Silu,
                                         bias=t1[:], scale=s1[:])
                    t0 += nn
            nc.vector.memset(y1[:, 0:1], 0.0)
            nc.vector.memset(y1[:, NPY - 1:NPY], 0.0)

            # =============== stage B: cv2 ===============
            with tc.tile_pool(name="psB", bufs=3, space="PSUM") as psB:
                t0 = 0
                while t0 < NPY:
                    nn = min(512, NPY - t0)
                    for m in range(2):
                        ps = psB.tile([128, 512], f32)
                        nc.tensor.matmul(ps[:, :nn], w2t[:, 128 * m:128 * m + 128],
                                         y1[:, t0:t0 + nn], start=True, stop=True)
                        nc.scalar.activation(y[:, m, YOFF + t0:YOFF + t0 + nn],
                                             ps[:, :nn], AF.Silu,
                                             bias=t2[:, m:m + 1], scale=s2[:, m:m + 1])
                    t0 += nn
            y1pool.release()
            # zero the y head/tail and pad columns; zero out-of-image rows via rowmask
            for m in range(2):
                nc.vector.memset(y[:, m, 0:YOFF], 0.0)
                nc.vector.memset(y[:, m, YOFF + NPY:YPAD], 0.0)
                yv = AP(y.tensor, y[:].offset + m * YPAD + YOFF,
                        [[2 * YPAD, 128], [W, HY], [1, 2]])
                nc.vector.memset(yv, 0.0)
                yv2 = AP(y.tensor, y[:].offset + m * YPAD + YOFF + 82,
                         [[2 * YPAD, 128], [W, HY], [1, 2]])
                nc.vector.memset(yv2, 0.0)
                nc.vector.tensor_scalar(y[:, m, YOFF:YOFF + 2 * W],
                                        y[:, m, YOFF:YOFF + 2 * W],
                                        rowm[:, 0:1], None, OP.mult)
                nc.vector.tensor_scalar(y[:, m, YOFF + NPY - 2 * W:YOFF + NPY],
                                        y[:, m, YOFF + NPY - 2 * W:YOFF + NPY],
                                        rowm[:, 1:2], None, OP.mult)

            # =============== stage D: depthwise conv + LN + GELU ===============
            # D1: depthwise conv (diag matmuls) + per-pixel channel sums /
            # sumsq via N=1 matmuls. LN stats are finalized per half (tiles
            # [0,12) and [12,27)) so the DRAM bounce overlaps D1's tail.
            rb_row = P.tile([1, 2, NTOT], bf16)
            with tc.tile_pool(name="psD", bufs=3, space="PSUM") as psD, \
                 tc.tile_pool(name="psS", bufs=2, space="PSUM") as psS, \
                 tc.tile_pool(name="dtmp", bufs=2) as dtmp, \
                 tc.tile_pool(name="stt", bufs=2) as sttp, \
                 tc.tile_pool(name="psST", bufs=2, space="PSUM") as psST:

                def emit_d2_half(h0, h1):
                    nh = h1 - h0
                    st0 = AP(statb.tensor, statb[:].offset + 2 * h0,
                             [[NT * 2, 128], [2, nh]])
                    st1 = AP(statb.tensor, statb[:].offset + 2 * h0 + 1,
                             [[NT * 2, 128], [2, nh]])
                    meanb = sttp.tile([128, 16], f32, tag="meanb")
                    nc.vector.tensor_scalar(meanb[:, :nh], st0, 1.0 / 256, None, OP.mult)
                    ex2 = sttp.tile([128, 16], f32, tag="ex2")
                    nc.vector.tensor_scalar(ex2[:, :nh], st1, 1.0 / 256, None, OP.mult)
                    msq = sttp.tile([128, 16], f32, tag="msq")
                    nc.vector.tensor_tensor(msq[:, :nh], meanb[:, :nh], meanb[:, :nh],
                                            OP.mult)
                    nc.vector.tensor_tensor(ex2[:, :nh], ex2[:, :nh], msq[:, :nh],
                                            OP.subtract)
                    sdev = sttp.tile([128, 16], f32, tag="sdev")
                    nc.scalar.activation(sdev[:, :nh], ex2[:, :nh], AF.Sqrt,
                                         bias=eps128[:], scale=1.0)
                    rstdb = sttp.tile([128, 16], f32, tag="rstdb")
                    with nc.allow_low_precision(reason="LN rstd"):
                        nc.vector.reciprocal(rstdb[:, :nh], sdev[:, :nh])
                    browb = sttp.tile([128, 16], f32, tag="browb")
                    nc.vector.scalar_tensor_tensor(browb[:, :nh], meanb[:, :nh], -1.0,
                                                   rstdb[:, :nh], OP.mult, OP.mult)
                    rbT = sttp.tile([16, 256], bf16, tag="rbT")
                    for src, col0 in ((rstdb, 0), (browb, 128)):
                        pT = psST.tile([128, 128], f32, tag="pT")
                        nc.tensor.transpose(pT[:nh, :], src[:, :nh], ident[:])
                        nc.vector.tensor_copy(rbT[:nh, col0:col0 + 128], pT[:nh, :])
                    st_dma = nc.sync.dma_start(
                        d_rbs.ap()[h0:h1],
                        rbT[:nh, :].rearrange("p (a b) -> p a b", a=2))
                    for a in range(2):
                        ld_dma = nc.sync.dma_start(
                            rb_row[:, a, 128 * h0:128 * h1].rearrange(
                                "o (b c) -> o b c", b=nh),
                            d_rbs.ap()[h0:h1, a].unsqueeze(0))
                        tile.add_dep_helper(ld_dma.ins, st_dma.ins,
                                            reason="dram bounce ordering")

                t0 = 0
                while t0 < NTOT:
                    nn = min(512, NTOT - t0)
                    sc = nn // 128
                    sq = [None, None]
                    for m in range(2):
                        ps = psD.tile([128, 512], f32, tag="pdw")
                        for ss in range(9):
                            ki, kj = ss // 3, ss % 3
                            off = (ki + 1) * W + kj - 1 + YOFF
                            nc.tensor.matmul(ps[:, :nn], dwdg[:, m, ss, :],
                                             y[:, m, t0 + off: t0 + off + nn],
                                             start=(ss == 0), stop=(ss == 8))
                        nc.vector.tensor_scalar(dw[:, m, t0:t0 + nn], ps[:, :nn],
                                                dwb[:, m:m + 1], None, OP.add)
                        sqm = dtmp.tile([128, 512], bf16, tag=f"sq{m}")
                        nc.scalar.activation(sqm[:, :nn], dw[:, m, t0:t0 + nn],
                                             AF.Square)
                        sq[m] = sqm
                    pst = psS.tile([128, 8], f32, tag="pstat")
                    for sub in range(sc):
                        sl = slice(t0 + 128 * sub, t0 + 128 * sub + 128)
                        for m in range(2):
                            nc.tensor.matmul(pst[:, 2 * sub:2 * sub + 1], dw[:, m, sl],
                                             ones_col[:], start=(m == 0), stop=(m == 1))
                            nc.tensor.matmul(pst[:, 2 * sub + 1:2 * sub + 2],
                                             sq[m][:, 128 * sub:128 * sub + 128],
                                             ones_col[:], start=(m == 0), stop=(m == 1))
                    nc.vector.tensor_copy(statb[:, t0 // 128: t0 // 128 + sc, :],
                                          pst[:, :2 * sc])
                    t0 += nn
                    if t0 == 1536:
                        emit_d2_half(0, 12)
                emit_d2_half(12, NT)

            # D3: normalize + gelu (per-pixel rstd/brow broadcast via K=1 matmul)
            with tc.tile_pool(name="psAB", bufs=2, space="PSUM") as psAB, \
                 tc.tile_pool(name="dtmp2", bufs=2) as dtmp2:
                t0 = 0
                while t0 < NTOT:
                    nn = min(512, NTOT - t0)
                    psa = psAB.tile([128, 512], f32, tag="pa")
                    nc.tensor.matmul(psa[:, :nn], ones_row[:, 0:128],
                                     rb_row[:, 0, t0:t0 + nn], start=True, stop=True)
                    psb = psAB.tile([128, 512], f32, tag="pb")
                    nc.tensor.matmul(psb[:, :nn], ones_row[:, 0:128],
                                     rb_row[:, 1, t0:t0 + nn], start=True, stop=True)
                    for m in range(2):
                        zt = dtmp2.tile([128, 512], bf16, tag=f"zt{m}")
                        nc.vector.tensor_tensor(zt[:, :nn], dw[:, m, t0:t0 + nn],
                                                psa[:, :nn], OP.mult)
                        nc.vector.tensor_tensor(zt[:, :nn], zt[:, :nn], psb[:, :nn], OP.add)
                        nc.scalar.activation(dw[:, m, t0:t0 + nn], zt[:, :nn],
                                             AF.Gelu, bias=lnb[:, m:m + 1], scale=lng[:, m:m + 1])
                    t0 += nn

            # =============== stage C: Xc projection -> xpm (pixel-major) ===============
            # (placed after D3 so its PE work fills the DVE-bound middle region)
            nc.vector.memset(xpm[:, 0, :], 0.0)
            nc.vector.memset(xpm[:, XCH - 1, :], 0.0)
            with tc.tile_pool(name="psC", bufs=3, space="PSUM") as psC:
                for Q in range(1, XCH - 1):
                    p0 = 128 * (Q - 1)
                    ps = psC.tile([128, 256], f32)
                    nc.tensor.matmul(ps[:], y[:, 0, p0:p0 + 128], wct[:, 0, :],
                                     start=True, stop=False)
                    nc.tensor.matmul(ps[:], y[:, 1, p0:p0 + 128], wct[:, 1, :],
                                     start=False, stop=False)
                    nc.tensor.matmul(ps[:], ones_row[:, 0:128], bcr[:],
                                     start=False, stop=True)
                    nc.vector.tensor_scalar(xpm[:, Q, :], ps[:],
                                            vmask[:, Q:Q + 1], None, OP.mult)

            # =============== stage E: offset/mask projection + coefficients ===============
            with tc.tile_pool(name="psE", bufs=3, space="PSUM") as psE:
                for T in range(NT):
                    ps = psE.tile([128, 32], f32)
                    nc.tensor.matmul(ps[:], dw[:, 0, 128 * T:128 * T + 128], womt[:, 0, :],
                                     start=True, stop=False)
                    nc.tensor.matmul(ps[:], dw[:, 1, 128 * T:128 * T + 128], womt[:, 1, :],
                                     start=False, stop=True)
                    nc.vector.tensor_tensor(coefb[:, T, :], ps[:], ombb[:], OP.add)

            # coefficient math, vectorized over all tiles: views [128, NT, 9]
            cf = coefb[:].offset
            cten = coefb.tensor

            def cview(col0, step, cnt=9):
                return AP(cten, cf + col0, [[NT * 32, 128], [32, NT], [step, cnt]])

            ox = cview(0, 2)
            oy = cview(1, 2)
            lg = cview(18, 1)
            with tc.tile_pool(name="cf", bufs=1) as cfp:
                mx = cfp.tile([128, NT], f32)
                nc.vector.tensor_reduce(mx[:], lg, axis=AX.X, op=OP.max)
                E = cfp.tile([128, NT, 9], f32)
                mxb = AP(mx.tensor, mx[:].offset, [[NT, 128], [1, NT], [0, 9]])
                nc.vector.tensor_tensor(E[:], lg, mxb, OP.subtract)
                nc.scalar.activation(E[:], E[:], AF.Exp)
                se = cfp.tile([128, NT], f32)
                nc.vector.tensor_reduce(se[:], E[:], axis=AX.X, op=OP.add)
                rs = cfp.tile([128, NT], f32)
                nc.vector.reciprocal(rs[:], se[:])
                msm = cfp.tile([128, NT, 9], f32)
                rsb = AP(rs.tensor, rs[:].offset, [[NT, 128], [1, NT], [0, 9]])
                nc.vector.tensor_tensor(msm[:], E[:], rsb, OP.mult)
                # fractional parts and floor indicators
                ix = cfp.tile([128, NT, 9], f32)
                nc.vector.tensor_scalar(ix[:], ox, 0.0, None, OP.is_lt)
                iy = cfp.tile([128, NT, 9], f32)
                nc.vector.tensor_scalar(iy[:], oy, 0.0, None, OP.is_lt)
                lx = cfp.tile([128, NT, 9], f32)
                nc.vector.tensor_tensor(lx[:], ox, ix[:], OP.add)
                ly = cfp.tile([128, NT, 9], f32)
                nc.vector.tensor_tensor(ly[:], oy, iy[:], OP.add)
                wx0 = cfp.tile([128, NT, 9], f32)
                nc.vector.tensor_scalar(wx0[:], lx[:], -1.0, 1.0, OP.mult, OP.add)
                wy0 = cfp.tile([128, NT, 9], f32)
                nc.vector.tensor_scalar(wy0[:], ly[:], -1.0, 1.0, OP.mult, OP.add)
                mx0 = cfp.tile([128, NT, 9], f32)
                nc.vector.tensor_scalar(mx0[:], ix[:], -1.0, 1.0, OP.mult, OP.add)
                my0 = cfp.tile([128, NT, 9], f32)
                nc.vector.tensor_scalar(my0[:], iy[:], -1.0, 1.0, OP.mult, OP.add)
                ta = cfp.tile([128, NT, 9], f32)
                nc.vector.tensor_tensor(ta[:], msm[:], wy0[:], OP.mult)
                tb = cfp.tile([128, NT, 9], f32)
                nc.vector.tensor_tensor(tb[:], msm[:], ly[:], OP.mult)
                pab = []
                for a, tv in ((0, ta), (1, tb)):
                    for b, wv in ((0, wx0), (1, lx)):
                        pv = cfp.tile([128, NT, 9], f32, name=f"p{a}{b}")
                        nc.vector.tensor_tensor(pv[:], tv[:], wv[:], OP.mult)
                        pab.append((a, b, pv))
                nc.vector.memset(cbuf[:], 0.0)
                gt = cfp.tile([128, NT, 9], f32)
                contrib = cfp.tile([128, NT, 9], f32)
                for sy, myv in ((0, my0), (1, iy)):
                    for sx, mxv in ((0, mx0), (1, ix)):
                        nc.vector.tensor_tensor(gt[:], myv[:], mxv[:], OP.mult)
                        for a, b, pv in pab:
                            u0 = 1 + a - sy
                            v0 = 1 + b - sx
                            nc.vector.tensor_tensor(contrib[:], pv[:], gt[:], OP.mult)
                            # C5[:, :, u0 + j, v0 + i] += contrib[i, j]
                            dstv = AP(cbuf.tensor, cbuf[:].offset + (u0 * 5 + v0),
                                      [[NT * 26, 128], [26, NT], [1, 3], [5, 3]])
                            srcv = AP(contrib.tensor, contrib[:].offset,
                                      [[NT * 9, 128], [9, NT], [3, 3], [1, 3]])
                            nc.vector.tensor_tensor(dstv, dstv, srcv, OP.add)
                nc.vector.tensor_copy(cbufh[:], cbuf[:])

            # =============== stage F: sampling + finalize ===============
            with tc.tile_pool(name="mtp", bufs=3) as mtp, \
                 tc.tile_pool(name="msb", bufs=3) as msbp, \
                 tc.tile_pool(name="psT", bufs=2, space="PSUM") as psT, \
                 tc.tile_pool(name="psZ", bufs=2, space="PSUM") as psZ, \
                 tc.tile_pool(name="fin", bufs=3) as fin:
                for T in range(NT):
                    mt = mtp.tile([128, MTW], bf16, tag="mt")
                    nc.gpsimd.local_scatter(mt[:].bitcast(i16), cbufh[:, T, :].bitcast(i16),
                                            sidx[:], channels=128, num_elems=MTW,
                                            num_idxs=26)
                    msb = msbp.tile([128, NCI, 128], bf16, tag="msb")
                    if USE_DMA_TRANSPOSE:
                        eng = nc.sync if T % 2 == 0 else nc.scalar
                        eng.dma_start(msb[:], mt[:], transpose=True)
                    else:
                        for ci in range(NCI):
                            pst = psT.tile([128, 128], bf16, tag="pst")
                            nc.tensor.transpose(pst[:], mt[:, 128 * ci:128 * ci + 128],
                                                identb[:])
                            if ci % 2 == 0:
                                nc.vector.tensor_copy(msb[:, ci, :], pst[:])
                            else:
                                nc.scalar.copy(msb[:, ci, :], pst[:])
                    psz = psZ.tile([128, 256], f32, tag="psz")
                    for ci in range(NCI):
                        nc.tensor.matmul(psz[:], msb[:, ci, :], xpm[:, T + 1 + ci, :],
                                         start=(ci == 0), stop=(ci == NCI - 1))
                    zt3 = fin.tile([128, 256], f32, tag="zt3")
                    nc.vector.tensor_tensor(zt3[:], psz[:], t3b[:], OP.add)
                    osb = fin.tile([128, 256], f32, tag="osb")
                    nc.scalar.activation(osb[:], zt3[:], AF.Silu)
                    nc.sync.dma_start(d_out.ap()[T], osb[:])

            if dump:
                nc.sync.dma_start(d_dy.ap(), y[:].rearrange("p a b -> p (a b)"))
                nc.sync.dma_start(d_ddwg.ap(), dw[:].rearrange("p a b -> p (a b)"))
                nc.sync.dma_start(d_dxpm.ap(), xpm[:].rearrange("p a b -> p (a b)"))
                nc.sync.dma_start(d_dcoef.ap(), coefb[:].rearrange("p a b -> p (a b)"))
                nc.sync.dma_start(d_dcbuf.ap(), cbufh[:].rearrange("p a b -> p (a b)"))
                nc.sync.dma_start(d_drb.ap(), rb_row[:].rearrange("o a b -> o (a b)"))
                nc.sync.dma_start(d_dstat.ap(), statb[:].rearrange("p a b -> p (a b)"))

    nc.compile()
    return nc


def _get_built():
    global _BUILT
    if _BUILT is None:
        _BUILT = _build()
    return _BUILT


def _prep(inputs):
    g = {k: np.asarray(v, dtype=np.float32) for k, v in inputs.items()}
    x = g["x"]
    b16 = ml_dtypes.bfloat16

    s1 = g["g1"] / np.sqrt(g["v1"] + EPS_BN)
    t1 = g["b1"] - g["m1"] * s1
    s2 = g["g2"] / np.sqrt(g["v2"] + EPS_BN)
    t2 = g["b2"] - g["m2"] * s2
    s3 = g["g3"] / np.sqrt(g["v3"] + EPS_BN)
    t3 = g["b3"] - g["m3"] * s3

    w1 = g["w1"]  # [128, 256, 3, 3]
    w1t = np.zeros((9, 2, 128, 128), np.float32)
    for ki in range(3):
        for kj in range(3):
            for ck in range(2):
                w1t[ki * 3 + kj, ck] = w1[:, 128 * ck:128 * ck + 128, ki, kj].T
    w2t = g["w2"][:, :, 0, 0].T.copy()  # [128, 256]
    Wc = g["out_w"] @ g["in_w"]
    wct = np.stack([Wc.T[:128], Wc.T[128:]])  # [2, 128, 256]
    bc = (g["out_w"] @ g["in_b"] + g["out_b"])[None, :]  # [1, 256]
    dwdg = np.zeros((2, 9, 128, 128), np.float32)
    for ck in range(2):
        for s in range(9):
            np.fill_diagonal(dwdg[ck, s], g["dw_w"][128 * ck:128 * ck + 128, 0, s // 3, s % 3])
    dwb = np.ascontiguousarray(g["dw_b"].reshape(2, 128).T).astype(np.float32)
    womt = np.zeros((2, 128, 32), np.float32)
    for ck in range(2):
        womt[ck, :, :18] = g["off_w"][:, 128 * ck:128 * ck + 128].T
        womt[ck, :, 18:27] = g["msk_w"][:, 128 * ck:128 * ck + 128].T
    omb = np.zeros((1, 32), np.float32)
    omb[0, :18] = g["off_b"]
    omb[0, 18:27] = g["msk_b"]
    ident = np.eye(128, dtype=np.float32)

    def colsplit(v):  # [256] -> [128, 2]
        return np.ascontiguousarray(v.reshape(2, 128).T).astype(np.float32)

    # fold BN3 scale into the combined projection; t3 added on-chip
    wct = (wct.reshape(2, 128, 256) * s3[None, None, :]).astype(np.float32)
    bc = (bc * s3[None, :]).astype(np.float32)

    shared = dict(
        onesr=np.ones((1, 512), b16),
        onesc=np.ones((128, 1), b16),
        w1t=np.ascontiguousarray(w1t.transpose(2, 0, 1, 3)).astype(b16),
        w2t=w2t.astype(b16),
        wct=np.ascontiguousarray(wct.transpose(1, 0, 2)).astype(b16),
        bcr=bc.astype(b16),
        dwdg=np.ascontiguousarray(dwdg.transpose(2, 0, 1, 3)).astype(b16),
        dwb=dwb,
        womt=np.ascontiguousarray(womt.transpose(1, 0, 2)).astype(b16),
        omb=omb.astype(b16),
        s1=s1[:, None], t1=t1[:, None],
        s2=colsplit(s2), t2=colsplit(t2),
        lng=colsplit(g["ln_g"]), lnb=colsplit(g["ln_b"]), ident=ident,
        identb=ident.astype(b16),
        t3r=t3[None, :].astype(b16),
    )

    in_maps = []
    for c in range(8):
        n, h = c // 2, c % 2
        r0 = 40 * h - 3  # x frame row 0 in global coords
        xs = np.zeros((2, 128, HX, W), np.float32)
        glo = max(r0, 0)
        ghi = min(r0 + HX, 80)
        xs[0, :, glo - r0:ghi - r0, 2:82] = x[n, :128, glo:ghi, :]
        xs[1, :, glo - r0:ghi - r0, 2:82] = x[n, 128:, glo:ghi, :]
        # validity mask for xpm pixels: q = 128*Q + p, pix = q - LEAD
        vm = np.zeros((XCH * 128,), np.float32)
        qs = np.arange(XCH * 128)
        pix = qs - LEAD
        rv, cv = pix // W, pix % W
        gr = 40 * h + rv - 2
        ok = (pix >= 0) & (pix < NPY) & (cv >= 2) & (cv < 82) & (gr >= 0) & (gr < 80)
        vm[ok] = 1.0
        vmask = vm.reshape(XCH, 128).T.copy()  # [128, XCH]
        rowm = np.zeros((128, 2), np.float32)
        rowm[:, 0] = 0.0 if h == 0 else 1.0   # y rows [0,2) valid only for h=1
        rowm[:, 1] = 1.0 if h == 0 else 0.0   # y rows [42,44) valid only for h=0
        m = dict(shared)
        m["xs"] = np.ascontiguousarray(
            xs.reshape(2, 128, NPX).transpose(1, 0, 2)).astype(b16)
        m["vmask"] = vmask
        m["rowm"] = rowm
        in_maps.append(m)
    return in_maps


def kernel(**inputs):
    nc = _get_built()
    in_maps = _prep(inputs)
    res = bass_utils.run_bass_kernel_spmd(nc, in_maps, core_ids=list(range(8)))
    x = np.asarray(inputs["x"], np.float32)
    out = np.zeros((4, 256, 80, 80), np.float32)
    for c in range(8):
        n, h = c // 2, c % 2
        o = np.asarray(res.results[c]["out"], np.float32).reshape(NT * 128, 256)[:NPD]
        o = o.reshape(HD, W, 256)[:, 2:82].transpose(2, 0, 1)
        out[n, :, 40 * h:40 * h + 40, :] = o
    out += x
    return out
